# revision 35
# baseline (speedup 1.0000x reference)
"""DINO loss kernel for Trainium2 (8 NeuronCores, Bass/Tile).

Math
----
Reference computes, with q = log_softmax(student/ts) [Ns=1280, D] and
p = softmax((teacher-center)/tt) [Nt=256, D]:

    loss = sum_{i != j} ( -sum_d p[i,d] q[j,d] ) / (Nt*Ns - Nt)

The full-pair sum factorizes over d:

    sum_{i,j} ce[i,j] = -sum_d P[d] * Q[d]
      P[d] = sum_i p[i,d]                (teacher prob column sums)
      Q[d] = sum_j q[j,d] = S[d]/ts - C  (S = raw student logit column sums,
                                          C = sum_j logsumexp_j(x/ts))
    diag  = sum_i p_i . q_i  (the i == j terms, to be subtracted)

    loss = ( -(dot(P,S)/ts - C*sum(P)) + diag ) / (Nt*Ns - Nt)

The device does only streaming reductions over the [N, D] inputs (no
[Nt,Ns,D] einsum): per-row sum-exp stats (for C), raw column sums S, and
1/Z-weighted teacher-prob column sums P. The tiny diag correction
(O(Nt*D), ~0.1% of the flops) and all cross-core merging run on the host
in float64.

Sharding (8 cores)
------------------
Pure data parallel over rows, one NEFF run, no collectives:
  core c gets student_local rows [128c,128c+128)           -> sl  [128, 65536]
           student_global rows [32c,32c+32) row-split x4   -> sg  [128, 16384]
           teacher rows        [32c,32c+32) row-split x4   -> t   [128, 16384]
Row-split x4: row i of a [32, 65536] slice is spread over partitions
4i..4i+3, 16384 columns each (a plain reshape(128, 16384) on the host),
so all engines run at full 128-partition width.

Implementation notes
--------------------
The kernel sits on the ACT-engine roofline: 12.6M exps/core at 128
lanes/cycle is ~82us, and the schedule keeps ACT >92% busy end to end.
Everything else is arranged around that:

* All big inputs travel as bf16 (halves HBM/DMA traffic to ~70us/core,
  under the ACT floor; end-to-end quantization error is ~1e-4 against a
  2e-2 gate). The teacher is exp'd in place in bf16; p normalization
  (e/Z) uses consistently rounded values so the ratios stay exact.
* Both student AND teacher exps use host-sampled upper bounds (sample
  max + margin) as the common exp bias. Softmax ratios are shift-
  invariant, so the bound only has to avoid overflow -- elements far
  below it underflow to 0 and contribute ~e^-60 relative, i.e. nothing.
  This removes any on-device row-max pass. If a bound is breached
  (outlier the sampler missed), stats come back non-finite and kernel()
  falls back to an exact numpy evaluation.
* Column sums run on the PE as mask-weighted bf16 matmuls (1 cyc/row,
  4-row PSUM bank tiles). The Z fold across the 4 partitions of each
  teacher row is ONE block-diagonal matmul (gmask) that also broadcasts
  the folded sums back to all 128 partitions -- deliberately no DMA: a
  tiny compute-dependent DMA would queue behind every prefetched load
  on the single DMA-engine bundle and stall the P matmuls ~30us.
* DMA queue discipline (the big scheduling win over the previous
  version): the SP/sync queue carries ONLY input loads, in consumption
  order -- a waiting DMA holds its sequencer, so no compute-dependent
  DMA may ever sit in front of a load. PSUM banks retire through DVE
  copies into bf16 SBUF stage tiles (fast WAR release: the PE never
  waits on a DMA round-trip) and ship to DRAM in big batches on the
  Pool/SWDGE queue (which never touches the loads' HWDGE path).
* The teacher arrives in 5 pieces with a small first piece so the ACT
  exp stream starts at ~4us; a warm-up exp pulls the ACT table load
  into the initial DMA latency shadow.
"""

import numpy as np
import ml_dtypes

import concourse.bass as bass
import concourse.bacc as bacc
import concourse.tile as tile
from concourse import mybir
from concourse.bass_utils import run_bass_kernel_spmd

F32 = mybir.dt.float32
BF16 = mybir.dt.bfloat16
AX = mybir.AxisListType
EXP = mybir.ActivationFunctionType.Exp
BF = ml_dtypes.bfloat16

N_CORES = 8
D = 65536
N_T = 256
N_G = 256
N_L = 1024
SL_ROWS = N_L // N_CORES          # 128 student_local rows per core
SG_ROWS = N_G // N_CORES          # 32 student_global rows per core
T_ROWS = N_T // N_CORES           # 32 teacher rows per core


def _masks(P=128):
    # qmask[p, m] = 1 if m == p % 4: out row m = colsum over partition group
    # m (row-split x4 quarter colsums, weighted later by 1/Z for P).
    qmask = np.zeros((P, 4), BF)
    qmask[np.arange(P), np.arange(P) % 4] = 1.0
    # emask block q ([:, 4q:4q+4]) has ones only in column q: lhsT that adds
    # a plain colsum of quarter q into row q of a 4-row PSUM region.
    emask = np.zeros((P, 16), BF)
    for q in range(4):
        emask[:, 4 * q + q] = 1.0
    # gmask[p', p] = 1 iff p'//4 == p//4: one matmul folds the 4 per-quarter
    # partial Z's of each logical row and broadcasts to all 4 partitions.
    gmask = np.zeros((P, P), BF)
    for r in range(P // 4):
        gmask[4 * r : 4 * r + 4, 4 * r : 4 * r + 4] = 1.0
    return qmask, emask, gmask


def build_nc(D=D, n_sl_chunks=8, ts=0.1, tt=0.04):
    """Build the per-core Bass program. All 8 cores run this same NEFF."""
    DQ = D // 4                    # 16384 columns per quarter
    CQ = DQ // n_sl_chunks         # 2048 sl quarter-cols per DMA chunk
    reg = 512                      # matmul free size (one PSUM bank of f32)
    bank_n = 2 * reg               # quarter-cols per PSUM bank tile
    assert CQ % bank_n == 0
    nb = DQ // bank_n              # bank tiles per full stream (sg/p)
    cht = DQ // 4                  # teacher/sg DMA+exp chunk size

    nc = bacc.Bacc()
    sl = nc.dram_tensor("sl", [128, D], BF16, kind="ExternalInput")
    sg = nc.dram_tensor("sg", [128, DQ], BF16, kind="ExternalInput")
    t = nc.dram_tensor("t", [128, DQ], BF16, kind="ExternalInput")
    nb2 = nc.dram_tensor("nb2", [128, 2], F32, kind="ExternalInput")

    qmask_np, emask_np, gmask_np = _masks()
    masks_d = nc.inline_tensor(
        np.concatenate([qmask_np, emask_np, gmask_np], axis=1), name="masks_c")

    s_sl = nc.dram_tensor("s_sl", [4, DQ], BF16, kind="ExternalOutput")
    s_sg = nc.dram_tensor("s_sg", [4, DQ], BF16, kind="ExternalOutput")
    p_out = nc.dram_tensor("p_out", [4, DQ], BF16, kind="ExternalOutput")
    w_sl = nc.dram_tensor("w_sl", [128, n_sl_chunks], F32, kind="ExternalOutput")
    w_sg = nc.dram_tensor("w_sg", [128, 4], F32, kind="ExternalOutput")

    with tile.TileContext(nc) as tc:
        with (
            tc.tile_pool(name="singles", bufs=1) as singles,
            tc.tile_pool(name="big", bufs=1) as big,
            tc.tile_pool(name="chunks", bufs=3) as chunks,
            tc.tile_pool(name="escr", bufs=1) as escr,
            tc.tile_pool(name="stats", bufs=1) as stats,
            tc.tile_pool(name="stage", bufs=6) as stage_pool,
            tc.tile_pool(name="psA", bufs=2, space="PSUM") as psA,
            tc.tile_pool(name="psB", bufs=3, space="PSUM") as psB,
            tc.tile_pool(name="psC", bufs=1, space="PSUM") as psC,
        ):
            # The SP/sync queue carries ONLY input loads, in consumption
            # order. Nothing on it ever waits on compute.
            tr = big.tile([128, DQ], BF16)
            sgr = big.tile([128, DQ], BF16)
            # Teacher arrives in 5 pieces, small first, so the ACT exp
            # stream starts as early as the DMA latency allows (~4.5us).
            tch = [(0, 2048), (2048, 2048), (4096, 4096),
                   (8192, 4096), (12288, 4096)]
            nc.sync.dma_start(out=tr[:, 0:2048], in_=t[:, 0:2048])
            nb2_t = singles.tile([128, 2], F32)
            nc.sync.dma_start(out=nb2_t, in_=nb2[:, :])
            nbs_t = nb2_t[:, 0:1]
            ntb_t = nb2_t[:, 1:2]
            lo, n = tch[1]
            nc.sync.dma_start(out=tr[:, lo : lo + n], in_=t[:, lo : lo + n])
            masks = singles.tile([128, 148], BF16)
            nc.sync.dma_start(out=masks, in_=masks_d[:, :])
            qmask = masks[:, 0:4]
            emask = masks[:, 4:20]
            gmask = masks[:, 20:148]
            for lo, n in tch[2:]:
                nc.sync.dma_start(out=tr[:, lo : lo + n], in_=t[:, lo : lo + n])
            for j in range(4):
                nc.sync.dma_start(
                    out=sgr[:, j * cht : (j + 1) * cht],
                    in_=sg[:, j * cht : (j + 1) * cht],
                )

            # Warm the ACT exp table while the first teacher piece loads.
            warm = stats.tile([128, 1], F32)
            nc.vector.memset(warm, 0.0)
            nc.scalar.activation(warm, warm, EXP)

            # teacher exp (in place, bf16) + per-partition partial Z sums
            zT = stats.tile([128, len(tch)], F32)
            for j, (lo, n) in enumerate(tch):
                nc.scalar.activation(
                    tr[:, lo : lo + n],
                    tr[:, lo : lo + n],
                    EXP, bias=ntb_t, scale=1.0 / tt,
                    accum_out=zT[:, j : j + 1],
                )

            # Z fold across the 4 partitions of each logical teacher row:
            # one block-diagonal matmul broadcasts the group sums back to
            # every partition (NO DMA -- a tiny compute-dependent DMA here
            # would queue behind all prefetched loads on the DMA engines).
            zloc = stats.tile([128, 1], BF16)
            with nc.allow_low_precision(reason="Z fold feeds a bf16 matmul; "
                                        "0.4% on Z is far inside tolerance"):
                nc.vector.reduce_sum(zloc, zT, axis=AX.X)
            zfold = psC.tile([128, 1], F32)
            nc.tensor.matmul(zfold, gmask, zloc, start=True, stop=True)
            rzb = stats.tile([128, 1], F32)
            nc.vector.reciprocal(rzb, zfold)
            wq = stats.tile([128, 4], BF16)
            nc.vector.tensor_scalar_mul(wq, qmask, rzb)

            # student_global exp stats (scratch out; sgr stays raw; same
            # host-supplied bound as student_local).
            wG = stats.tile([128, 4], F32)

            def sg_exp(j):
                sc = escr.tile([128, cht], BF16, tag="escr")
                nc.scalar.activation(
                    sc, sgr[:, j * cht : (j + 1) * cht],
                    EXP, bias=nbs_t, scale=1.0 / ts,
                    accum_out=wG[:, j : j + 1],
                )

            # PSUM retire: bank -> SBUF stage slot on DVE (fast WAR release
            # so the PE never waits on a DMA round-trip); one Pool/SWDGE DMA
            # stores each filled [4, 4096] stage tile. Big stage groups +
            # deep buffering so store transfers queueing behind prefetched
            # loads on the DMA engines never back-propagates to the PE.
            def make_stream(dst, bank_cols, group):
                state = {"st": None, "n0": 0, "cnt": 0}

                def add(bank, bank_i):
                    if state["st"] is None:
                        st = stage_pool.tile(
                            [4, group * bank_cols], BF16, tag="stage",
                            name="st")
                        state["st"] = st
                        state["n0"] = bank_i
                        state["cnt"] = 0
                    q = state["cnt"]
                    with nc.allow_low_precision(
                            reason="bf16 colsum outputs; 0.4% per column is "
                            "far inside the 2e-2 gate"):
                        nc.vector.tensor_copy(
                            out=state["st"][:, q * bank_cols
                                            : (q + 1) * bank_cols],
                            in_=bank)
                    state["cnt"] += 1
                    if state["cnt"] == group:
                        nc.gpsimd.dma_start(
                            out=dst[:, state["n0"] * bank_cols
                                    : (state["n0"] + group) * bank_cols],
                            in_=state["st"])
                        state["st"] = None
                return add

            sg_ret = make_stream(s_sg, bank_n, 4)
            p_ret = make_stream(p_out, bank_n, 4)
            sl_ret = make_stream(s_sl, reg, 8)

            def _abank(lhsT, srct, ret, bank_i):
                bank = psA.tile([4, bank_n], F32, tag="bankA")
                for s in range(bank_n // reg):
                    lo = bank_i * bank_n + s * reg
                    nc.tensor.matmul(
                        bank[:, s * reg : (s + 1) * reg],
                        lhsT,
                        srct[:, lo : lo + reg],
                        start=True, stop=True,
                    )
                ret(bank, bank_i)

            def sg_bank(bank_i):
                _abank(qmask, sgr, sg_ret, bank_i)

            def p_bank(bank_i):
                _abank(wq, tr, p_ret, bank_i)

            # student_local stream: chunk DMA (sync queue), colsum matmuls
            # (PE, [4, reg] banks), exp on ACT (throwaway scratch + row-sum
            # accumulator), retire via the shared stage/store path.
            slv = sl.rearrange("p (q k c) -> p q k c", q=4, k=n_sl_chunks)
            wS = stats.tile([128, n_sl_chunks], F32)

            def sl_chunk(k):
                ch = chunks.tile([128, 4, CQ], BF16, tag="chunk")
                nc.sync.dma_start(out=ch, in_=slv[:, :, k, :])
                sc = escr.tile([128, 4 * CQ], BF16, tag="escr")
                nc.scalar.activation(
                    sc.rearrange("p (q c) -> p q c", q=4), ch, EXP,
                    bias=nbs_t, scale=1.0 / ts,
                    accum_out=wS[:, k : k + 1],
                )
                for b in range(CQ // reg):
                    bank = psB.tile([4, reg], F32, tag="bankB")
                    cl = b * reg
                    for q in range(4):
                        nc.tensor.matmul(
                            bank,
                            emask[:, 4 * q : 4 * q + 4],
                            ch[:, q, cl : cl + reg],
                            start=(q == 0),
                            stop=(q == 3),
                        )
                    sl_ret(bank, k * (CQ // reg) + b)

            # ---- interleaved schedule (per-engine order == emission; the
            # tile scheduler may refine within dependency limits) ----
            for j in range(4):
                sg_exp(j)
            for i in range(nb):
                sg_bank(i)
            sl_chunk(0)
            for i in range(0, 4):
                p_bank(i)
            sl_chunk(1)
            for i in range(4, 8):
                p_bank(i)
            sl_chunk(2)
            for i in range(8, 12):
                p_bank(i)
            sl_chunk(3)
            for i in range(12, 16):
                p_bank(i)
            for k in range(4, n_sl_chunks):
                sl_chunk(k)

            # stats out: w_sg rides the Pool queue (ready mid-stream); w_sl
            # (gated by the final sl exp) goes at the very end of ACT's own
            # queue so it cannot head-of-line-block the Pool stores.
            nc.gpsimd.dma_start(out=w_sg[:, :], in_=wG)
            nc.scalar.dma_start(out=w_sl[:, :], in_=wS)

    nc.compile()
    return nc


_NC_CACHE = {}


def _get_nc(ts, tt):
    key = (round(ts, 9), round(tt, 9))
    if key not in _NC_CACHE:
        _NC_CACHE[key] = build_nc(ts=ts, tt=tt)
    return _NC_CACHE[key]


def _merge(results, ts, tt, bs_scaled, diag1, n_sl_chunks=8):
    """Host-side exact merge of per-core device outputs (float64).

    bs_scaled = b_s/ts, the (already scaled) exp bound the device used for
    student rows; diag1 = sum_i v_i/(ts*Z_i), computed exactly on the host
    (O(Nt*D), ~0.1% of the kernel flops). Returns (loss, healthy).
    """
    S = np.zeros(D, np.float64)
    P = np.zeros(D, np.float64)
    C = 0.0       # sum of all student row logsumexps
    C_g = 0.0     # global-student-row portion
    healthy = True
    for r in results:
        S += r["s_sl"].astype(np.float64).reshape(-1)
        S += r["s_sg"].astype(np.float64).reshape(-1)
        P += r["p_out"].astype(np.float64).reshape(-1)
        # student_local rows: common bound -> lse = b/ts + log(sum w)
        w = r["w_sl"].astype(np.float64)               # [128, nch]
        wsum = w.sum(axis=1)
        healthy &= bool(np.isfinite(w).all() and (wsum > 0).all())
        C += (bs_scaled + np.log(np.maximum(wsum, 1e-300))).sum()
        # student_global rows: common bound per-partition lse -> merge 4s
        wg = r["w_sg"].astype(np.float64).sum(axis=1)  # [128]
        healthy &= bool(np.isfinite(wg).all() and (wg > 0).all())
        lp = (bs_scaled + np.log(np.maximum(wg, 1e-300))).reshape(32, 4)
        mxg = lp.max(axis=1, keepdims=True)
        lse_g = mxg[:, 0] + np.log(np.exp(lp - mxg).sum(axis=1))
        C += lse_g.sum()
        C_g += lse_g.sum()
        healthy &= bool(np.isfinite(r["s_sl"]).all()
                        and np.isfinite(r["s_sg"]).all()
                        and np.isfinite(r["p_out"]).all())

    cross = P @ S / ts - C * P.sum()
    diag = diag1 - C_g
    total = -cross + diag
    n_s = N_G + N_L
    n_loss_terms = N_T * n_s - min(N_T, n_s)
    loss = total / n_loss_terms
    healthy &= bool(np.isfinite(loss))
    return loss, healthy


def _numpy_loss(sg_full, sl_full, teacher, ts, tt):
    """Exact host fallback (never hit for sane input distributions)."""
    x = np.concatenate([sg_full, sl_full], axis=0).astype(np.float64) / ts
    lq = x - x.max(axis=1, keepdims=True)
    lq -= np.log(np.exp(lq).sum(axis=1, keepdims=True))
    y = teacher.astype(np.float64) / tt
    e = np.exp(y - y.max(axis=1, keepdims=True))
    p = e / e.sum(axis=1, keepdims=True)
    ce = -(p @ lq.T)
    n_t, n_s = ce.shape
    idx = np.arange(n_t)
    ce[idx, idx] = 0.0
    return ce.sum() / (n_t * n_s - min(n_t, n_s))


def kernel(out_student_global, out_student_local, out_teacher, center,
           temp_student, temp_teacher, cent_rate_m):
    out_student_global = np.asarray(out_student_global)
    out_student_local = np.asarray(out_student_local)
    out_teacher = np.asarray(out_teacher)
    center = np.asarray(center)
    ts = float(np.asarray(temp_student).reshape(-1)[0])
    tt = float(np.asarray(temp_teacher).reshape(-1)[0])

    teacher = out_teacher
    if np.any(center):
        teacher = out_teacher - center.reshape(1, -1).astype(np.float32)
    teacher = np.ascontiguousarray(teacher, dtype=np.float32)
    sg_full = np.ascontiguousarray(out_student_global, dtype=np.float32)
    sl_full = np.ascontiguousarray(out_student_local, dtype=np.float32)

    t_bf = teacher.astype(BF)
    sg_bf = sg_full.astype(BF)
    sl_bf = sl_full.astype(BF)

    # Safe exp bounds: strided-sample max + margin. Softmax ratios are
    # shift-invariant, so only overflow matters (margin << 88*temp).
    smax = max(float(sl_full.ravel()[::257].max()),
               float(sg_full.ravel()[::257].max()))
    b_s = smax + 1.0
    b_t = float(teacher.ravel()[::257].max()) + 2.0
    nb2 = np.empty((128, 2), np.float32)
    nb2[:, 0] = -b_s / ts
    nb2[:, 1] = -b_t / tt

    # Diagonal term sum_i p_i . (sg_i/ts): exact f64 on the host -- O(Nt*D)
    # is ~0.1% of the kernel's flops and removes a whole device pass.
    y = teacher.astype(np.float64) / tt
    y -= y.max(axis=1, keepdims=True)
    e = np.exp(y)
    diag1 = float(
        ((e * sg_full.astype(np.float64)).sum(axis=1) / e.sum(axis=1)).sum()
        / ts)

    nc = _get_nc(ts, tt)
    in_maps = []
    for c in range(N_CORES):
        in_maps.append({
            "sl": sl_bf[c * SL_ROWS:(c + 1) * SL_ROWS],
            "sg": sg_bf[c * SG_ROWS:(c + 1) * SG_ROWS].reshape(128, D // 4),
            "t": t_bf[c * T_ROWS:(c + 1) * T_ROWS].reshape(128, D // 4),
            "nb2": nb2,
        })
    res = run_bass_kernel_spmd(nc, in_maps, core_ids=list(range(N_CORES)))
    loss, healthy = _merge(res.results, ts, tt, b_s / ts, diag1)
    if not healthy:
        loss = _numpy_loss(sg_full, sl_full, teacher, ts, tt)
    return np.float32(loss)


# revision 37
# speedup vs baseline: 1.0015x; 1.0015x over previous
"""DINO loss kernel for Trainium2 (8 NeuronCores, Bass/Tile).

Math
----
Reference computes, with q = log_softmax(student/ts) [Ns=1280, D] and
p = softmax((teacher-center)/tt) [Nt=256, D]:

    loss = sum_{i != j} ( -sum_d p[i,d] q[j,d] ) / (Nt*Ns - Nt)

The full-pair sum factorizes over d:

    sum_{i,j} ce[i,j] = -sum_d P[d] * Q[d]
      P[d] = sum_i p[i,d]                (teacher prob column sums)
      Q[d] = sum_j q[j,d] = S[d]/ts - C  (S = raw student logit column sums,
                                          C = sum_j logsumexp_j(x/ts))
    diag  = sum_i p_i . q_i  (the i == j terms, to be subtracted)

    loss = ( -(dot(P,S)/ts - C*sum(P)) + diag ) / (Nt*Ns - Nt)

The device does only streaming reductions over the [N, D] inputs (no
[Nt,Ns,D] einsum): per-row sum-exp stats (for C), raw column sums S, and
1/Z-weighted teacher-prob column sums P. The tiny diag correction
(O(Nt*D), ~0.1% of the flops) and all cross-core merging run on the host
in float64.

Sharding (8 cores)
------------------
Pure data parallel over rows, one NEFF run, no collectives:
  core c gets student_local rows [128c,128c+128)           -> sl  [128, 65536]
           student_global rows [32c,32c+32) row-split x4   -> sg  [128, 16384]
           teacher rows        [32c,32c+32) row-split x4   -> t   [128, 16384]
Row-split x4: row i of a [32, 65536] slice is spread over partitions
4i..4i+3, 16384 columns each (a plain reshape(128, 16384) on the host),
so all engines run at full 128-partition width.

Implementation notes
--------------------
The kernel sits on the ACT-engine roofline: 12.6M exps/core at 128
lanes/cycle is ~82us, and the schedule keeps ACT >92% busy end to end.
Everything else is arranged around that:

* All big inputs travel as bf16 (halves HBM/DMA traffic to ~70us/core,
  under the ACT floor; end-to-end quantization error is ~1e-4 against a
  2e-2 gate). The teacher is exp'd in place in bf16; p normalization
  (e/Z) uses consistently rounded values so the ratios stay exact.
* Both student AND teacher exps use host-sampled upper bounds (sample
  max + margin) as the common exp bias. Softmax ratios are shift-
  invariant, so the bound only has to avoid overflow -- elements far
  below it underflow to 0 and contribute ~e^-60 relative, i.e. nothing.
  This removes any on-device row-max pass. If a bound is breached
  (outlier the sampler missed), stats come back non-finite and kernel()
  falls back to an exact numpy evaluation.
* Column sums run on the PE as mask-weighted bf16 matmuls (1 cyc/row,
  4-row PSUM bank tiles). The Z fold across the 4 partitions of each
  teacher row is ONE block-diagonal matmul (gmask) that also broadcasts
  the folded sums back to all 128 partitions -- deliberately no DMA: a
  tiny compute-dependent DMA would queue behind every prefetched load
  on the single DMA-engine bundle and stall the P matmuls ~30us.
* DMA queue discipline (the big scheduling win over the previous
  version): the SP/sync queue carries ONLY input loads, in consumption
  order -- a waiting DMA holds its sequencer, so no compute-dependent
  DMA may ever sit in front of a load. PSUM banks retire through DVE
  copies into bf16 SBUF stage tiles (fast WAR release: the PE never
  waits on a DMA round-trip) and ship to DRAM in big batches on the
  Pool/SWDGE queue (which never touches the loads' HWDGE path).
* The teacher arrives in 5 pieces with a small first piece so the ACT
  exp stream starts at ~4us; a warm-up exp pulls the ACT table load
  into the initial DMA latency shadow.
"""

import numpy as np
import ml_dtypes

import concourse.bass as bass
import concourse.bacc as bacc
import concourse.tile as tile
from concourse import mybir
from concourse.bass_utils import run_bass_kernel_spmd

F32 = mybir.dt.float32
BF16 = mybir.dt.bfloat16
AX = mybir.AxisListType
EXP = mybir.ActivationFunctionType.Exp
BF = ml_dtypes.bfloat16

N_CORES = 8
D = 65536
N_T = 256
N_G = 256
N_L = 1024
SL_ROWS = N_L // N_CORES          # 128 student_local rows per core
SG_ROWS = N_G // N_CORES          # 32 student_global rows per core
T_ROWS = N_T // N_CORES           # 32 teacher rows per core


def _masks(P=128):
    # qmask[p, m] = 1 if m == p % 4: out row m = colsum over partition group
    # m (row-split x4 quarter colsums, weighted later by 1/Z for P).
    qmask = np.zeros((P, 4), BF)
    qmask[np.arange(P), np.arange(P) % 4] = 1.0
    # emask block q ([:, 4q:4q+4]) has ones only in column q: lhsT that adds
    # a plain colsum of quarter q into row q of a 4-row PSUM region.
    emask = np.zeros((P, 16), BF)
    for q in range(4):
        emask[:, 4 * q + q] = 1.0
    # gmask[p', p] = 1 iff p'//4 == p//4: one matmul folds the 4 per-quarter
    # partial Z's of each logical row and broadcasts to all 4 partitions.
    gmask = np.zeros((P, P), BF)
    for r in range(P // 4):
        gmask[4 * r : 4 * r + 4, 4 * r : 4 * r + 4] = 1.0
    return qmask, emask, gmask


def build_nc(D=D, n_sl_chunks=8, ts=0.1, tt=0.04):
    """Build the per-core Bass program. All 8 cores run this same NEFF."""
    DQ = D // 4                    # 16384 columns per quarter
    CQ = DQ // n_sl_chunks         # 2048 sl quarter-cols per DMA chunk
    reg = 512                      # matmul free size (one PSUM bank of f32)
    bank_n = 2 * reg               # quarter-cols per PSUM bank tile
    assert CQ % bank_n == 0
    nb = DQ // bank_n              # bank tiles per full stream (sg/p)
    cht = DQ // 4                  # teacher/sg DMA+exp chunk size

    nc = bacc.Bacc()
    sl = nc.dram_tensor("sl", [128, D], BF16, kind="ExternalInput")
    sg = nc.dram_tensor("sg", [128, DQ], BF16, kind="ExternalInput")
    t = nc.dram_tensor("t", [128, DQ], BF16, kind="ExternalInput")
    nb2 = nc.dram_tensor("nb2", [128, 2], F32, kind="ExternalInput")

    qmask_np, emask_np, gmask_np = _masks()
    masks_d = nc.inline_tensor(
        np.concatenate([qmask_np, emask_np, gmask_np], axis=1), name="masks_c")

    s_sl = nc.dram_tensor("s_sl", [4, DQ], BF16, kind="ExternalOutput")
    s_sg = nc.dram_tensor("s_sg", [4, DQ], BF16, kind="ExternalOutput")
    p_out = nc.dram_tensor("p_out", [4, DQ], BF16, kind="ExternalOutput")
    w_sl = nc.dram_tensor("w_sl", [128, n_sl_chunks], F32, kind="ExternalOutput")
    w_sg = nc.dram_tensor("w_sg", [128, 4], F32, kind="ExternalOutput")

    with tile.TileContext(nc) as tc:
        with (
            tc.tile_pool(name="singles", bufs=1) as singles,
            tc.tile_pool(name="big", bufs=1) as big,
            tc.tile_pool(name="chunks", bufs=3) as chunks,
            tc.tile_pool(name="escr", bufs=1) as escr,
            tc.tile_pool(name="stats", bufs=1) as stats,
            tc.tile_pool(name="stage", bufs=6) as stage_pool,
            tc.tile_pool(name="psA", bufs=2, space="PSUM") as psA,
            tc.tile_pool(name="psB", bufs=3, space="PSUM") as psB,
            tc.tile_pool(name="psC", bufs=1, space="PSUM") as psC,
        ):
            # The SP/sync queue carries ONLY input loads, in consumption
            # order. Nothing on it ever waits on compute.
            tr = big.tile([128, DQ], BF16)
            sgr = big.tile([128, DQ], BF16)
            # Teacher arrives in 5 pieces, small first, so the ACT exp
            # stream starts as early as the DMA latency allows (~4.5us).
            tch = [(0, 2048), (2048, 2048), (4096, 4096),
                   (8192, 4096), (12288, 4096)]
            nc.sync.dma_start(out=tr[:, 0:2048], in_=t[:, 0:2048])
            nb2_t = singles.tile([128, 2], F32)
            nc.sync.dma_start(out=nb2_t, in_=nb2[:, :])
            nbs_t = nb2_t[:, 0:1]
            ntb_t = nb2_t[:, 1:2]
            lo, n = tch[1]
            nc.sync.dma_start(out=tr[:, lo : lo + n], in_=t[:, lo : lo + n])
            masks = singles.tile([128, 148], BF16)
            nc.sync.dma_start(out=masks, in_=masks_d[:, :])
            qmask = masks[:, 0:4]
            emask = masks[:, 4:20]
            gmask = masks[:, 20:148]
            for lo, n in tch[2:]:
                nc.sync.dma_start(out=tr[:, lo : lo + n], in_=t[:, lo : lo + n])
            for j in range(4):
                nc.sync.dma_start(
                    out=sgr[:, j * cht : (j + 1) * cht],
                    in_=sg[:, j * cht : (j + 1) * cht],
                )

            # Warm the ACT exp table while the first teacher piece loads.
            warm = stats.tile([128, 1], F32)
            nc.vector.memset(warm, 0.0)
            nc.scalar.activation(warm, warm, EXP)

            # teacher exp (in place, bf16) + per-partition partial Z sums
            zT = stats.tile([128, len(tch)], F32)
            for j, (lo, n) in enumerate(tch):
                nc.scalar.activation(
                    tr[:, lo : lo + n],
                    tr[:, lo : lo + n],
                    EXP, bias=ntb_t, scale=1.0 / tt,
                    accum_out=zT[:, j : j + 1],
                )

            # Z fold across the 4 partitions of each logical teacher row:
            # one block-diagonal matmul broadcasts the group sums back to
            # every partition (NO DMA -- a tiny compute-dependent DMA here
            # would queue behind all prefetched loads on the DMA engines).
            zloc = stats.tile([128, 1], BF16)
            with nc.allow_low_precision(reason="Z fold feeds a bf16 matmul; "
                                        "0.4% on Z is far inside tolerance"):
                nc.vector.reduce_sum(zloc, zT, axis=AX.X)
            zfold = psC.tile([128, 1], F32)
            nc.tensor.matmul(zfold, gmask, zloc, start=True, stop=True)
            rzb = stats.tile([128, 1], F32)
            nc.vector.reciprocal(rzb, zfold)
            wq = stats.tile([128, 4], BF16)
            nc.vector.tensor_scalar_mul(wq, qmask, rzb)

            # student_global exp stats (scratch out; sgr stays raw; same
            # host-supplied bound as student_local).
            wG = stats.tile([128, 4], F32)

            def sg_exp(j):
                sc = escr.tile([128, cht], BF16, tag="escr")
                nc.scalar.activation(
                    sc, sgr[:, j * cht : (j + 1) * cht],
                    EXP, bias=nbs_t, scale=1.0 / ts,
                    accum_out=wG[:, j : j + 1],
                )

            # PSUM retire: bank -> SBUF stage slot on DVE (fast WAR release
            # so the PE never waits on a DMA round-trip); one Pool/SWDGE DMA
            # stores each filled [4, 4096] stage tile. Big stage groups +
            # deep buffering so store transfers queueing behind prefetched
            # loads on the DMA engines never back-propagates to the PE.
            def make_stream(dst, bank_cols, group):
                state = {"st": None, "n0": 0, "cnt": 0}

                def add(bank, bank_i):
                    if state["st"] is None:
                        st = stage_pool.tile(
                            [4, group * bank_cols], BF16, tag="stage",
                            name="st")
                        state["st"] = st
                        state["n0"] = bank_i
                        state["cnt"] = 0
                    q = state["cnt"]
                    with nc.allow_low_precision(
                            reason="bf16 colsum outputs; 0.4% per column is "
                            "far inside the 2e-2 gate"):
                        nc.vector.tensor_copy(
                            out=state["st"][:, q * bank_cols
                                            : (q + 1) * bank_cols],
                            in_=bank)
                    state["cnt"] += 1
                    if state["cnt"] == group:
                        nc.gpsimd.dma_start(
                            out=dst[:, state["n0"] * bank_cols
                                    : (state["n0"] + group) * bank_cols],
                            in_=state["st"])
                        state["st"] = None
                return add

            sg_ret = make_stream(s_sg, bank_n, 4)
            p_ret = make_stream(p_out, bank_n, 4)
            sl_ret = make_stream(s_sl, reg, 8)

            def _abank(lhsT, srct, ret, bank_i):
                bank = psA.tile([4, bank_n], F32, tag="bankA")
                for s in range(bank_n // reg):
                    lo = bank_i * bank_n + s * reg
                    nc.tensor.matmul(
                        bank[:, s * reg : (s + 1) * reg],
                        lhsT,
                        srct[:, lo : lo + reg],
                        start=True, stop=True,
                    )
                ret(bank, bank_i)

            def sg_bank(bank_i):
                _abank(qmask, sgr, sg_ret, bank_i)

            def p_bank(bank_i):
                _abank(wq, tr, p_ret, bank_i)

            # student_local stream: chunk DMA (sync queue), colsum matmuls
            # (PE, [4, reg] banks), exp on ACT (throwaway scratch + row-sum
            # accumulator), retire via the shared stage/store path.
            slv = sl.rearrange("p (q k c) -> p q k c", q=4, k=n_sl_chunks)
            wS = stats.tile([128, n_sl_chunks], F32)

            def sl_chunk(k):
                ch = chunks.tile([128, 4, CQ], BF16, tag="chunk")
                nc.sync.dma_start(out=ch, in_=slv[:, :, k, :])
                sc = escr.tile([128, 4 * CQ], BF16, tag="escr")
                nc.scalar.activation(
                    sc.rearrange("p (q c) -> p q c", q=4), ch, EXP,
                    bias=nbs_t, scale=1.0 / ts,
                    accum_out=wS[:, k : k + 1],
                )
                for b in range(CQ // reg):
                    bank = psB.tile([4, reg], F32, tag="bankB")
                    cl = b * reg
                    for q in range(4):
                        nc.tensor.matmul(
                            bank,
                            emask[:, 4 * q : 4 * q + 4],
                            ch[:, q, cl : cl + reg],
                            start=(q == 0),
                            stop=(q == 3),
                        )
                    sl_ret(bank, k * (CQ // reg) + b)

            # ---- interleaved schedule (per-engine order == emission; the
            # tile scheduler may refine within dependency limits) ----
            for j in range(4):
                sg_exp(j)
            for i in range(nb):
                sg_bank(i)
            sl_chunk(0)
            for i in range(0, 4):
                p_bank(i)
            sl_chunk(1)
            for i in range(4, 8):
                p_bank(i)
            sl_chunk(2)
            for i in range(8, 12):
                p_bank(i)
            sl_chunk(3)
            for i in range(12, 16):
                p_bank(i)
            for k in range(4, n_sl_chunks):
                sl_chunk(k)

            # stats out: w_sg rides the Pool queue (ready mid-stream); w_sl
            # (gated by the final sl exp) goes at the very end of ACT's own
            # queue so it cannot head-of-line-block the Pool stores.
            nc.gpsimd.dma_start(out=w_sg[:, :], in_=wG)
            nc.sync.dma_start(out=w_sl[:, :], in_=wS)

    nc.compile()
    return nc


_NC_CACHE = {}


def _get_nc(ts, tt):
    key = (round(ts, 9), round(tt, 9))
    if key not in _NC_CACHE:
        _NC_CACHE[key] = build_nc(ts=ts, tt=tt)
    return _NC_CACHE[key]


def _merge(results, ts, tt, bs_scaled, diag1, n_sl_chunks=8):
    """Host-side exact merge of per-core device outputs (float64).

    bs_scaled = b_s/ts, the (already scaled) exp bound the device used for
    student rows; diag1 = sum_i v_i/(ts*Z_i), computed exactly on the host
    (O(Nt*D), ~0.1% of the kernel flops). Returns (loss, healthy).
    """
    S = np.zeros(D, np.float64)
    P = np.zeros(D, np.float64)
    C = 0.0       # sum of all student row logsumexps
    C_g = 0.0     # global-student-row portion
    healthy = True
    for r in results:
        S += r["s_sl"].astype(np.float64).reshape(-1)
        S += r["s_sg"].astype(np.float64).reshape(-1)
        P += r["p_out"].astype(np.float64).reshape(-1)
        # student_local rows: common bound -> lse = b/ts + log(sum w)
        w = r["w_sl"].astype(np.float64)               # [128, nch]
        wsum = w.sum(axis=1)
        healthy &= bool(np.isfinite(w).all() and (wsum > 0).all())
        C += (bs_scaled + np.log(np.maximum(wsum, 1e-300))).sum()
        # student_global rows: common bound per-partition lse -> merge 4s
        wg = r["w_sg"].astype(np.float64).sum(axis=1)  # [128]
        healthy &= bool(np.isfinite(wg).all() and (wg > 0).all())
        lp = (bs_scaled + np.log(np.maximum(wg, 1e-300))).reshape(32, 4)
        mxg = lp.max(axis=1, keepdims=True)
        lse_g = mxg[:, 0] + np.log(np.exp(lp - mxg).sum(axis=1))
        C += lse_g.sum()
        C_g += lse_g.sum()
        healthy &= bool(np.isfinite(r["s_sl"]).all()
                        and np.isfinite(r["s_sg"]).all()
                        and np.isfinite(r["p_out"]).all())

    cross = P @ S / ts - C * P.sum()
    diag = diag1 - C_g
    total = -cross + diag
    n_s = N_G + N_L
    n_loss_terms = N_T * n_s - min(N_T, n_s)
    loss = total / n_loss_terms
    healthy &= bool(np.isfinite(loss))
    return loss, healthy


def _numpy_loss(sg_full, sl_full, teacher, ts, tt):
    """Exact host fallback (never hit for sane input distributions)."""
    x = np.concatenate([sg_full, sl_full], axis=0).astype(np.float64) / ts
    lq = x - x.max(axis=1, keepdims=True)
    lq -= np.log(np.exp(lq).sum(axis=1, keepdims=True))
    y = teacher.astype(np.float64) / tt
    e = np.exp(y - y.max(axis=1, keepdims=True))
    p = e / e.sum(axis=1, keepdims=True)
    ce = -(p @ lq.T)
    n_t, n_s = ce.shape
    idx = np.arange(n_t)
    ce[idx, idx] = 0.0
    return ce.sum() / (n_t * n_s - min(n_t, n_s))


def kernel(out_student_global, out_student_local, out_teacher, center,
           temp_student, temp_teacher, cent_rate_m):
    out_student_global = np.asarray(out_student_global)
    out_student_local = np.asarray(out_student_local)
    out_teacher = np.asarray(out_teacher)
    center = np.asarray(center)
    ts = float(np.asarray(temp_student).reshape(-1)[0])
    tt = float(np.asarray(temp_teacher).reshape(-1)[0])

    teacher = out_teacher
    if np.any(center):
        teacher = out_teacher - center.reshape(1, -1).astype(np.float32)
    teacher = np.ascontiguousarray(teacher, dtype=np.float32)
    sg_full = np.ascontiguousarray(out_student_global, dtype=np.float32)
    sl_full = np.ascontiguousarray(out_student_local, dtype=np.float32)

    t_bf = teacher.astype(BF)
    sg_bf = sg_full.astype(BF)
    sl_bf = sl_full.astype(BF)

    # Safe exp bounds: strided-sample max + margin. Softmax ratios are
    # shift-invariant, so only overflow matters (margin << 88*temp).
    smax = max(float(sl_full.ravel()[::257].max()),
               float(sg_full.ravel()[::257].max()))
    b_s = smax + 1.0
    b_t = float(teacher.ravel()[::257].max()) + 2.0
    nb2 = np.empty((128, 2), np.float32)
    nb2[:, 0] = -b_s / ts
    nb2[:, 1] = -b_t / tt

    # Diagonal term sum_i p_i . (sg_i/ts): exact f64 on the host -- O(Nt*D)
    # is ~0.1% of the kernel's flops and removes a whole device pass.
    y = teacher.astype(np.float64) / tt
    y -= y.max(axis=1, keepdims=True)
    e = np.exp(y)
    diag1 = float(
        ((e * sg_full.astype(np.float64)).sum(axis=1) / e.sum(axis=1)).sum()
        / ts)

    nc = _get_nc(ts, tt)
    in_maps = []
    for c in range(N_CORES):
        in_maps.append({
            "sl": sl_bf[c * SL_ROWS:(c + 1) * SL_ROWS],
            "sg": sg_bf[c * SG_ROWS:(c + 1) * SG_ROWS].reshape(128, D // 4),
            "t": t_bf[c * T_ROWS:(c + 1) * T_ROWS].reshape(128, D // 4),
            "nb2": nb2,
        })
    res = run_bass_kernel_spmd(nc, in_maps, core_ids=list(range(N_CORES)))
    loss, healthy = _merge(res.results, ts, tt, b_s / ts, diag1)
    if not healthy:
        loss = _numpy_loss(sg_full, sl_full, teacher, ts, tt)
    return np.float32(loss)


# revision 47
# speedup vs baseline: 1.0097x; 1.0082x over previous
"""DINO loss kernel for Trainium2 (8 NeuronCores, Bass/Tile).

Math
----
Reference computes, with q = log_softmax(student/ts) [Ns=1280, D] and
p = softmax((teacher-center)/tt) [Nt=256, D]:

    loss = sum_{i != j} ( -sum_d p[i,d] q[j,d] ) / (Nt*Ns - Nt)

The full-pair sum factorizes over d:

    sum_{i,j} ce[i,j] = -sum_d P[d] * Q[d]
      P[d] = sum_i p[i,d]                (teacher prob column sums)
      Q[d] = sum_j q[j,d] = S[d]/ts - C  (S = raw student logit column sums,
                                          C = sum_j logsumexp_j(x/ts))
    diag  = sum_i p_i . q_i  (the i == j terms, to be subtracted)

    loss = ( -(dot(P,S)/ts - C*sum(P)) + diag ) / (Nt*Ns - Nt)

The device does only streaming reductions over the [N, D] inputs (no
[Nt,Ns,D] einsum): per-row sum-exp stats (for C), raw column sums S, and
1/Z-weighted teacher-prob column sums P. The tiny diag correction
(O(Nt*D), ~0.1% of the flops) and all cross-core merging run on the host
in float64.

Sharding (8 cores)
------------------
Pure data parallel over rows, one NEFF run, no collectives:
  core c gets student_local rows [128c,128c+128)           -> sl  [128, 65536]
           student_global rows [32c,32c+32) row-split x4   -> sg  [128, 16384]
           teacher rows        [32c,32c+32) row-split x4   -> t   [128, 16384]
Row-split x4: row i of a [32, 65536] slice is spread over partitions
4i..4i+3, 16384 columns each (a plain reshape(128, 16384) on the host),
so all engines run at full 128-partition width.

Implementation notes
--------------------
The kernel sits on the ACT-engine roofline: 12.6M exps/core at 128
lanes/cycle is ~82us, and the schedule keeps ACT >92% busy end to end.
Everything else is arranged around that:

* All big inputs travel as bf16 (halves HBM/DMA traffic to ~70us/core,
  under the ACT floor; end-to-end quantization error is ~1e-4 against a
  2e-2 gate). The teacher is exp'd in place in bf16; p normalization
  (e/Z) uses consistently rounded values so the ratios stay exact.
* Both student AND teacher exps use host-sampled upper bounds (sample
  max + margin) as the common exp bias. Softmax ratios are shift-
  invariant, so the bound only has to avoid overflow -- elements far
  below it underflow to 0 and contribute ~e^-60 relative, i.e. nothing.
  This removes any on-device row-max pass. If a bound is breached
  (outlier the sampler missed), stats come back non-finite and kernel()
  falls back to an exact numpy evaluation.
* Column sums run on the PE as mask-weighted bf16 matmuls (1 cyc/row,
  4-row PSUM bank tiles). The Z fold across the 4 partitions of each
  teacher row is ONE block-diagonal matmul (gmask) that also broadcasts
  the folded sums back to all 128 partitions -- deliberately no DMA: a
  tiny compute-dependent DMA would queue behind every prefetched load
  on the single DMA-engine bundle and stall the P matmuls ~30us.
* DMA queue discipline (the big scheduling win over the previous
  version): the SP/sync queue carries ONLY input loads, in consumption
  order -- a waiting DMA holds its sequencer, so no compute-dependent
  DMA may ever sit in front of a load. PSUM banks retire through DVE
  copies into bf16 SBUF stage tiles (fast WAR release: the PE never
  waits on a DMA round-trip) and ship to DRAM in big batches on the
  Pool/SWDGE queue (which never touches the loads' HWDGE path).
* The teacher arrives in 5 pieces with a small first piece so the ACT
  exp stream starts at ~4us; a warm-up exp pulls the ACT table load
  into the initial DMA latency shadow.
"""

import numpy as np
import ml_dtypes

import concourse.bass as bass
import concourse.bacc as bacc
import concourse.tile as tile
from concourse import mybir
from concourse.bass_utils import run_bass_kernel_spmd

F32 = mybir.dt.float32
BF16 = mybir.dt.bfloat16
AX = mybir.AxisListType
EXP = mybir.ActivationFunctionType.Exp
BF = ml_dtypes.bfloat16

N_CORES = 8
D = 65536
N_T = 256
N_G = 256
N_L = 1024
SL_ROWS = N_L // N_CORES          # 128 student_local rows per core
SG_ROWS = N_G // N_CORES          # 32 student_global rows per core
T_ROWS = N_T // N_CORES           # 32 teacher rows per core


def _masks(P=128):
    # qmask[p, m] = 1 if m == p % 4: out row m = colsum over partition group
    # m (row-split x4 quarter colsums, weighted later by 1/Z for P).
    qmask = np.zeros((P, 4), BF)
    qmask[np.arange(P), np.arange(P) % 4] = 1.0
    # emask block q ([:, 4q:4q+4]) has ones only in column q: lhsT that adds
    # a plain colsum of quarter q into row q of a 4-row PSUM region.
    emask = np.zeros((P, 16), BF)
    for q in range(4):
        emask[:, 4 * q + q] = 1.0
    # gmask[p', p] = 1 iff p'//4 == p//4: one matmul folds the 4 per-quarter
    # partial Z's of each logical row and broadcasts to all 4 partitions.
    gmask = np.zeros((P, P), BF)
    for r in range(P // 4):
        gmask[4 * r : 4 * r + 4, 4 * r : 4 * r + 4] = 1.0
    return qmask, emask, gmask


def build_nc(D=D, n_sl_chunks=8, ts=0.1, tt=0.04):
    """Build the per-core Bass program. All 8 cores run this same NEFF."""
    DQ = D // 4                    # 16384 columns per quarter
    CQ = DQ // n_sl_chunks         # 2048 sl quarter-cols per DMA chunk
    reg = 512                      # matmul free size (one PSUM bank of f32)
    bank_n = 2 * reg               # quarter-cols per PSUM bank tile
    assert CQ % bank_n == 0
    nb = DQ // bank_n              # bank tiles per full stream (sg/p)
    cht = DQ // 4                  # teacher/sg DMA+exp chunk size

    nc = bacc.Bacc()
    sl = nc.dram_tensor("sl", [128, D], BF16, kind="ExternalInput")
    sg = nc.dram_tensor("sg", [128, DQ], BF16, kind="ExternalInput")
    t = nc.dram_tensor("t", [128, DQ], BF16, kind="ExternalInput")
    nb2 = nc.dram_tensor("nb2", [128, 2], F32, kind="ExternalInput")

    qmask_np, emask_np, gmask_np = _masks()
    masks_d = nc.inline_tensor(
        np.concatenate([qmask_np, emask_np, gmask_np], axis=1), name="masks_c")

    s_sl = nc.dram_tensor("s_sl", [4, DQ], BF16, kind="ExternalOutput")
    s_sg = nc.dram_tensor("s_sg", [4, DQ], BF16, kind="ExternalOutput")
    p_out = nc.dram_tensor("p_out", [4, DQ], BF16, kind="ExternalOutput")
    w_sl = nc.dram_tensor("w_sl", [128, n_sl_chunks], F32, kind="ExternalOutput")
    w_sg = nc.dram_tensor("w_sg", [128, 2], F32, kind="ExternalOutput")

    with tile.TileContext(nc) as tc:
        with (
            tc.tile_pool(name="singles", bufs=1) as singles,
            tc.tile_pool(name="big", bufs=1) as big,
            tc.tile_pool(name="chunks", bufs=3) as chunks,
            tc.tile_pool(name="escr", bufs=1) as escr,
            tc.tile_pool(name="stats", bufs=1) as stats,
            tc.tile_pool(name="stage", bufs=6) as stage_pool,
            tc.tile_pool(name="psA", bufs=2, space="PSUM") as psA,
            tc.tile_pool(name="psB", bufs=3, space="PSUM") as psB,
            tc.tile_pool(name="psC", bufs=1, space="PSUM") as psC,
        ):
            # The SP/sync queue carries ONLY input loads, in consumption
            # order. Nothing on it ever waits on compute.
            tr = big.tile([128, DQ], BF16)
            sgr = big.tile([128, DQ], BF16)
            # Teacher arrives in 5 pieces, small first, so the ACT exp
            # stream starts as early as the DMA latency allows (~4.5us).
            tch = [(0, 2048), (2048, 1792), (3840, 2304),
                   (6144, 4096), (10240, 6144)]
            nc.sync.dma_start(out=tr[:, 0:2048], in_=t[:, 0:2048])
            nb2_t = singles.tile([128, 2], F32)
            nc.sync.dma_start(out=nb2_t, in_=nb2[:, :])
            nbs_t = nb2_t[:, 0:1]
            ntb_t = nb2_t[:, 1:2]
            lo, n = tch[1]
            nc.sync.dma_start(out=tr[:, lo : lo + n], in_=t[:, lo : lo + n])
            masks = singles.tile([128, 148], BF16)
            nc.sync.dma_start(out=masks, in_=masks_d[:, :])
            qmask = masks[:, 0:4]
            emask = masks[:, 4:20]
            gmask = masks[:, 20:148]
            for lo, n in tch[2:]:
                nc.sync.dma_start(out=tr[:, lo : lo + n], in_=t[:, lo : lo + n])
            sgch = [(0, 7168), (7168, 9216)]
            for lo, n in sgch:
                nc.sync.dma_start(out=sgr[:, lo : lo + n], in_=sg[:, lo : lo + n])

            # Warm the ACT exp table while the first teacher piece loads.
            warm = stats.tile([128, 1], F32)
            nc.vector.memset(warm, 0.0)
            nc.scalar.activation(warm, warm, EXP)

            # teacher exp (in place, bf16) + per-partition partial Z sums
            zT = stats.tile([128, len(tch)], F32)
            for j, (lo, n) in enumerate(tch):
                nc.scalar.activation(
                    tr[:, lo : lo + n],
                    tr[:, lo : lo + n],
                    EXP, bias=ntb_t, scale=1.0 / tt,
                    accum_out=zT[:, j : j + 1],
                )

            # Z fold across the 4 partitions of each logical teacher row:
            # one block-diagonal matmul broadcasts the group sums back to
            # every partition (NO DMA -- a tiny compute-dependent DMA here
            # would queue behind all prefetched loads on the DMA engines).
            zloc = stats.tile([128, 1], BF16)
            with nc.allow_low_precision(reason="Z fold feeds a bf16 matmul; "
                                        "0.4% on Z is far inside tolerance"):
                nc.vector.reduce_sum(zloc, zT, axis=AX.X)
            zfold = psC.tile([128, 1], F32)
            nc.tensor.matmul(zfold, gmask, zloc, start=True, stop=True)
            rzb = stats.tile([128, 1], F32)
            nc.vector.reciprocal(rzb, zfold)
            wq = stats.tile([128, 4], BF16)
            nc.vector.tensor_scalar_mul(wq, qmask, rzb)

            # student_global exp stats (scratch out; sgr stays raw; same
            # host-supplied bound as student_local).
            wG = stats.tile([128, 2], F32)

            def sg_exp(j):
                lo, n = sgch[j]
                sc = escr.tile([128, 9216], BF16, tag="escr_sg")
                nc.scalar.activation(
                    sc[:, 0:n], sgr[:, lo : lo + n],
                    EXP, bias=nbs_t, scale=1.0 / ts,
                    accum_out=wG[:, j : j + 1],
                )

            # PSUM retire: bank -> SBUF stage slot on DVE (fast WAR release
            # so the PE never waits on a DMA round-trip); one Pool/SWDGE DMA
            # stores each filled [4, 4096] stage tile. Big stage groups +
            # deep buffering so store transfers queueing behind prefetched
            # loads on the DMA engines never back-propagates to the PE.
            def make_stream(dst, bank_cols, group):
                state = {"st": None, "n0": 0, "cnt": 0}

                def add(bank, bank_i):
                    if state["st"] is None:
                        st = stage_pool.tile(
                            [4, group * bank_cols], BF16, tag="stage",
                            name="st")
                        state["st"] = st
                        state["n0"] = bank_i
                        state["cnt"] = 0
                    q = state["cnt"]
                    with nc.allow_low_precision(
                            reason="bf16 colsum outputs; 0.4% per column is "
                            "far inside the 2e-2 gate"):
                        nc.vector.tensor_copy(
                            out=state["st"][:, q * bank_cols
                                            : (q + 1) * bank_cols],
                            in_=bank)
                    state["cnt"] += 1
                    if state["cnt"] == group:
                        nc.gpsimd.dma_start(
                            out=dst[:, state["n0"] * bank_cols
                                    : (state["n0"] + group) * bank_cols],
                            in_=state["st"])
                        state["st"] = None
                return add

            sg_ret = make_stream(s_sg, bank_n, 4)
            p_ret = make_stream(p_out, bank_n, 4)
            sl_ret = make_stream(s_sl, reg, 8)

            def _abank(lhsT, srct, ret, bank_i):
                bank = psA.tile([4, bank_n], F32, tag="bankA")
                for s in range(bank_n // reg):
                    lo = bank_i * bank_n + s * reg
                    nc.tensor.matmul(
                        bank[:, s * reg : (s + 1) * reg],
                        lhsT,
                        srct[:, lo : lo + reg],
                        start=True, stop=True,
                    )
                ret(bank, bank_i)

            def sg_bank(bank_i):
                _abank(qmask, sgr, sg_ret, bank_i)

            def p_bank(bank_i):
                _abank(wq, tr, p_ret, bank_i)

            # student_local stream: chunk DMA (sync queue), colsum matmuls
            # (PE, [4, reg] banks), exp on ACT (throwaway scratch + row-sum
            # accumulator), retire via the shared stage/store path.
            slv = sl.rearrange("p (q k c) -> p q k c", q=4, k=n_sl_chunks)
            wS = stats.tile([128, n_sl_chunks], F32)

            def sl_chunk(k):
                ch = chunks.tile([128, 4, CQ], BF16, tag="chunk")
                nc.sync.dma_start(out=ch, in_=slv[:, :, k, :])
                sc = escr.tile([128, 4 * CQ], BF16, tag="escr")
                nc.scalar.activation(
                    sc.rearrange("p (q c) -> p q c", q=4), ch, EXP,
                    bias=nbs_t, scale=1.0 / ts,
                    accum_out=wS[:, k : k + 1],
                )
                for b in range(CQ // reg):
                    bank = psB.tile([4, reg], F32, tag="bankB")
                    cl = b * reg
                    for q in range(4):
                        nc.tensor.matmul(
                            bank,
                            emask[:, 4 * q : 4 * q + 4],
                            ch[:, q, cl : cl + reg],
                            start=(q == 0),
                            stop=(q == 3),
                        )
                    sl_ret(bank, k * (CQ // reg) + b)

            # ---- interleaved schedule (per-engine order == emission; the
            # tile scheduler may refine within dependency limits) ----
            for j in range(2):
                sg_exp(j)
            for i in range(nb):
                sg_bank(i)
            sl_chunk(0)
            for i in range(0, 4):
                p_bank(i)
            sl_chunk(1)
            for i in range(4, 8):
                p_bank(i)
            sl_chunk(2)
            for i in range(8, 12):
                p_bank(i)
            sl_chunk(3)
            for i in range(12, 16):
                p_bank(i)
            for k in range(4, n_sl_chunks):
                sl_chunk(k)

            # stats out: w_sg rides the Pool queue (ready mid-stream); w_sl
            # (gated by the final sl exp) goes at the very end of ACT's own
            # queue so it cannot head-of-line-block the Pool stores.
            nc.gpsimd.dma_start(out=w_sg[:, :], in_=wG)
            nc.sync.dma_start(out=w_sl[:, :], in_=wS)

    nc.compile()
    return nc


_NC_CACHE = {}


def _get_nc(ts, tt):
    key = (round(ts, 9), round(tt, 9))
    if key not in _NC_CACHE:
        _NC_CACHE[key] = build_nc(ts=ts, tt=tt)
    return _NC_CACHE[key]


def _merge(results, ts, tt, bs_scaled, diag1, n_sl_chunks=8):
    """Host-side exact merge of per-core device outputs (float64).

    bs_scaled = b_s/ts, the (already scaled) exp bound the device used for
    student rows; diag1 = sum_i v_i/(ts*Z_i), computed exactly on the host
    (O(Nt*D), ~0.1% of the kernel flops). Returns (loss, healthy).
    """
    S = np.zeros(D, np.float64)
    P = np.zeros(D, np.float64)
    C = 0.0       # sum of all student row logsumexps
    C_g = 0.0     # global-student-row portion
    healthy = True
    for r in results:
        S += r["s_sl"].astype(np.float64).reshape(-1)
        S += r["s_sg"].astype(np.float64).reshape(-1)
        P += r["p_out"].astype(np.float64).reshape(-1)
        # student_local rows: common bound -> lse = b/ts + log(sum w)
        w = r["w_sl"].astype(np.float64)               # [128, nch]
        wsum = w.sum(axis=1)
        healthy &= bool(np.isfinite(w).all() and (wsum > 0).all())
        C += (bs_scaled + np.log(np.maximum(wsum, 1e-300))).sum()
        # student_global rows: common bound per-partition lse -> merge 4s
        wg = r["w_sg"].astype(np.float64).sum(axis=1)  # [128]
        healthy &= bool(np.isfinite(wg).all() and (wg > 0).all())
        lp = (bs_scaled + np.log(np.maximum(wg, 1e-300))).reshape(32, 4)
        mxg = lp.max(axis=1, keepdims=True)
        lse_g = mxg[:, 0] + np.log(np.exp(lp - mxg).sum(axis=1))
        C += lse_g.sum()
        C_g += lse_g.sum()
        healthy &= bool(np.isfinite(r["s_sl"]).all()
                        and np.isfinite(r["s_sg"]).all()
                        and np.isfinite(r["p_out"]).all())

    cross = P @ S / ts - C * P.sum()
    diag = diag1 - C_g
    total = -cross + diag
    n_s = N_G + N_L
    n_loss_terms = N_T * n_s - min(N_T, n_s)
    loss = total / n_loss_terms
    healthy &= bool(np.isfinite(loss))
    return loss, healthy


def _numpy_loss(sg_full, sl_full, teacher, ts, tt):
    """Exact host fallback (never hit for sane input distributions)."""
    x = np.concatenate([sg_full, sl_full], axis=0).astype(np.float64) / ts
    lq = x - x.max(axis=1, keepdims=True)
    lq -= np.log(np.exp(lq).sum(axis=1, keepdims=True))
    y = teacher.astype(np.float64) / tt
    e = np.exp(y - y.max(axis=1, keepdims=True))
    p = e / e.sum(axis=1, keepdims=True)
    ce = -(p @ lq.T)
    n_t, n_s = ce.shape
    idx = np.arange(n_t)
    ce[idx, idx] = 0.0
    return ce.sum() / (n_t * n_s - min(n_t, n_s))


def kernel(out_student_global, out_student_local, out_teacher, center,
           temp_student, temp_teacher, cent_rate_m):
    out_student_global = np.asarray(out_student_global)
    out_student_local = np.asarray(out_student_local)
    out_teacher = np.asarray(out_teacher)
    center = np.asarray(center)
    ts = float(np.asarray(temp_student).reshape(-1)[0])
    tt = float(np.asarray(temp_teacher).reshape(-1)[0])

    teacher = out_teacher
    if np.any(center):
        teacher = out_teacher - center.reshape(1, -1).astype(np.float32)
    teacher = np.ascontiguousarray(teacher, dtype=np.float32)
    sg_full = np.ascontiguousarray(out_student_global, dtype=np.float32)
    sl_full = np.ascontiguousarray(out_student_local, dtype=np.float32)

    t_bf = teacher.astype(BF)
    sg_bf = sg_full.astype(BF)
    sl_bf = sl_full.astype(BF)

    # Safe exp bounds: strided-sample max + margin. Softmax ratios are
    # shift-invariant, so only overflow matters (margin << 88*temp).
    smax = max(float(sl_full.ravel()[::257].max()),
               float(sg_full.ravel()[::257].max()))
    b_s = smax + 1.0
    b_t = float(teacher.ravel()[::257].max()) + 2.0
    nb2 = np.empty((128, 2), np.float32)
    nb2[:, 0] = -b_s / ts
    nb2[:, 1] = -b_t / tt

    # Diagonal term sum_i p_i . (sg_i/ts): exact f64 on the host -- O(Nt*D)
    # is ~0.1% of the kernel's flops and removes a whole device pass.
    y = teacher.astype(np.float64) / tt
    y -= y.max(axis=1, keepdims=True)
    e = np.exp(y)
    diag1 = float(
        ((e * sg_full.astype(np.float64)).sum(axis=1) / e.sum(axis=1)).sum()
        / ts)

    nc = _get_nc(ts, tt)
    in_maps = []
    for c in range(N_CORES):
        in_maps.append({
            "sl": sl_bf[c * SL_ROWS:(c + 1) * SL_ROWS],
            "sg": sg_bf[c * SG_ROWS:(c + 1) * SG_ROWS].reshape(128, D // 4),
            "t": t_bf[c * T_ROWS:(c + 1) * T_ROWS].reshape(128, D // 4),
            "nb2": nb2,
        })
    res = run_bass_kernel_spmd(nc, in_maps, core_ids=list(range(N_CORES)))
    loss, healthy = _merge(res.results, ts, tt, b_s / ts, diag1)
    if not healthy:
        loss = _numpy_loss(sg_full, sl_full, teacher, ts, tt)
    return np.float32(loss)
